# revision 61
# baseline (speedup 1.0000x reference)
"""Causal self-attention (B=4, T=2048, C=1024, H=16) on 8 TRN2 NeuronCores.

Sharding: data-parallel on batch (4) x tensor-parallel on heads (2 groups of
8). Core i handles batch i//2 and head-group i%2. Per core:
  - QKV matmuls for its head-group's weight columns. q,k are produced in
    transposed [feature, T] layout; v in natural [T, feature] layout with a
    ones column per head (sum(exp) accumulates in the attention matmul).
  - Causal attention per head-pair in scores^T layout [k, q]. No max
    subtraction: scores*hs^-0.5 are O(+-10), exp is safe. Fully-masked
    k-blocks are skipped; diagonal blocks are N-trimmed to the causal q-range
    and the remaining 128x128 triangle is masked with a DVE multiply against
    a precomputed 0/1 mask (NOT gpsimd affine_select - that serialized the
    collective queue and stalled the tail at half HAM clock).
  - The score->exp->att.v chain is software-pipelined 2 deep: PE emits
    st(i), ACT exp(i), PE av(i-2), so the PE never waits on the ACT engine.
    Filler work (V chunks, later pairs' q/k tiles, projection tiles) is
    injected every other iteration to keep the PE dense (HAM clock warm).
  - y^T is exchanged between the two cores of a batch with pairwise
    AllGathers per (pair, T-half); the last pair runs j-order [2,3,1,0] and
    splits its trailing half into two q-tile quarter AGs, so the final
    collective is small and lands right after the shortest j.
  - Projection accumulates in SBUF as AG chunks arrive (position-gated
    filler atoms with 3-deep yt DMA prefetch); b_proj folded in.
Host shuffles weights/biases into [p, ...]-contiguous layouts so every DMA
moves 1-8KB lines per partition (a bqk rearrange was a 1024-descriptor bomb);
x streams in T-quarter order so the first matmuls start after ~2.5MB.

dtypes: all matmul operands bf16; every accumulation fp32 in PSUM; softmax
normalization fp32 (measured ~5e-3 fro vs fp32 reference; gate is 2e-2).

Measured on HW: 345.6us exec (twice, +-15ns), vs 426-488us for the prior
session's baseline. PE active ~266us (~225us real work at 2.4GHz - the
scores/att.v matmuls are inherently ~50% PE-utilized at K=64/M=65);
remaining overheads: ~25us DMA-ring-bound startup, ~20us AG-bound tail,
HAM half-clock stretch around idle windows.

HW gotchas (CoreSim passes all of these; only real HW fails):
  - a single 65-partition DVE copy from PSUM silently corrupts data
  - gpsimd custom-DVE reciprocal misreads PSUM (stage rows in SBUF first)
  - junk "heater" matmuls and fine-grained (16x) AllGathers both regress
"""

import os
import sys
from contextlib import ExitStack

import numpy as np
import ml_dtypes

if "/opt/trn_rl_repo" not in sys.path:
    sys.path.insert(0, "/opt/trn_rl_repo")

import concourse.bass as bass
import concourse.mybir as mybir
import concourse.tile as tile
from concourse import bacc
from concourse import bass_utils

F32 = mybir.dt.float32
BF16 = mybir.dt.bfloat16
P = 128          # SBUF partitions
QT = 512         # q tile (matmul free dim)
KC = 128         # k chunk (psum partition dim)
HS = 64          # head size
KPQ = QT // KC   # k chunks per q tile

N_CORES = 8
PAIRS = [[0, 1], [2, 3], [4, 5], [6, 7]]

B_FULL, T_FULL, C_FULL, H_FULL = 4, 2048, 1024, 16


def build_nc(T=T_FULL, C=C_FULL, HL=H_FULL // 2):
    """Build the SPMD graph for one core (all 8 cores run the same graph).

    Per-core input tensors:
      xT    [C, T] bf16       x[b] transposed
      wqk   [2CL/P, P, C/P, P] bf16  w_attn q|k cols, host-shuffled [f,p,c,m]
      wv_s  [P, C/P, CL] bf16 w_attn v cols, host-shuffled [p,c,m]
      wp_s  [P, C/P, CL] bf16 w_proj cols for this core's output half
      bqk   [2*CL] f32, bv [CL] f32, bp [CL] f32
    Output: out [T, CL] f32.
    """
    CL = HL * HS                 # local width (q, k, v, out-cols each)
    n_cc = C // P                # x feature chunks (8)
    n_f = 2 * CL // P            # q|k f-tiles (4 q then 4 k)
    n_jt = T // QT               # q tiles (4)
    n_kt = T // KC               # k chunks / v t-chunks (16)
    n_pair = HL // 2             # head pairs (4)
    n_half = 2                   # T halves for AG chunking
    TH = T // n_half
    scale = HS ** -0.5

    nc = bacc.Bacc("TRN2", target_bir_lowering=False, debug=False,
                   num_devices=N_CORES)

    xT = nc.dram_tensor("xT", [C, T], BF16, kind="ExternalInput").ap()
    wqk = nc.dram_tensor("wqk", [n_f, P, n_cc, P], BF16,
                         kind="ExternalInput").ap()
    wv_s = nc.dram_tensor("wv_s", [P, n_cc, CL], BF16,
                          kind="ExternalInput").ap()
    wp_s = nc.dram_tensor("wp_s", [P, n_cc, CL], BF16,
                          kind="ExternalInput").ap()
    bqk = nc.dram_tensor("bqk", [P, 2 * CL // P], F32,
                         kind="ExternalInput").ap()
    bv = nc.dram_tensor("bv", [CL], F32, kind="ExternalInput").ap()
    bp = nc.dram_tensor("bp", [CL], F32, kind="ExternalInput").ap()
    out_ext = nc.dram_tensor("out", [T, CL], F32, kind="ExternalOutput").ap()

    with ExitStack() as ctx:
        tc = ctx.enter_context(tile.TileContext(nc))

        persist = ctx.enter_context(tc.tile_pool(name="persist", bufs=1))
        dram = ctx.enter_context(tc.tile_pool(name="dram", bufs=1, space="DRAM"))
        # st 2x2 banks + yp0 + yp1 + qps 2 = 8 banks
        ps = ctx.enter_context(tc.tile_pool(name="ps", bufs=1, space="PSUM"))
        att = ctx.enter_context(tc.tile_pool(name="att", bufs=1))

        # ---- persistent SBUF tiles -----------------------------------
        wqk_sb = [persist.tile([P, n_cc, P], BF16, tag=f"wqk{f}",
                               name=f"wqk{f}") for f in range(n_f)]
        wv_sb = persist.tile([P, n_cc, CL], BF16, tag="wv", name="wv")
        wp_sb = persist.tile([P, n_cc, CL], BF16, tag="wp", name="wp")
        x_sb = [persist.tile([P, T], BF16, tag=f"x{c}", name=f"x{c}")
                for c in range(n_cc)]
        qk_sb = [persist.tile([P, T], BF16, tag=f"qk{f}", name=f"qk{f}")
                 for f in range(n_f)]
        v_sb = [persist.tile([P, HL, HS + 2], BF16, tag=f"v{t}",
                             name=f"v{t}") for t in range(n_kt)]
        oacc = [persist.tile([P, CL], F32, tag=f"oacc{t}", name=f"oacc{t}")
                for t in range(T // P)]
        bqk_sb = persist.tile([P, n_f], F32, tag="bqk", name="bqk_sb")
        bv_bc = persist.tile([P, CL], F32, tag="bv_bc", name="bv_bc")
        bp_bc = persist.tile([P, CL], F32, tag="bp_bc", name="bp_bc")
        ones_f = persist.tile([P, HL, 1], F32, tag="ones_f", name="ones_f")
        # tri[p, g] = 1 where g >= p else 0 (keep-at-or-above-diagonal)
        tri = persist.tile([P, P], BF16, tag="tri", name="tri")

        # ---- input DMAs, in consumption order ------------------------
        # the three DGE rings (sync/gpsimd/scalar) give ~130GB/s aggregate;
        # order transfers so compute can start incrementally: weights for the
        # first atoms, then x by T-quarter so qk/V atoms unlock quarter by
        # quarter
        qs = [nc.sync, nc.gpsimd, nc.scalar]
        # tiny bias rows first (the first qk bias-add gates attention start;
        # bqk is host-shuffled to [p, f] so this is one descriptor per
        # partition, not a 1024-descriptor gather)
        nc.sync.dma_start(bqk_sb[:], bqk)
        bv_row = att.tile([1, CL], F32, tag="brow", bufs=2, name="bv_row")
        nc.sync.dma_start(bv_row[:], bv.rearrange("(o c) -> o c", o=1))
        bp_row = att.tile([1, CL], F32, tag="brow", bufs=2, name="bp_row")
        nc.sync.dma_start(bp_row[:], bp.rearrange("(o c) -> o c", o=1))
        nc.sync.dma_start(wqk_sb[0][:], wqk[0])
        nc.gpsimd.dma_start(wqk_sb[n_jt][:], wqk[n_jt])
        ri = 0
        for q4 in range(4):
            for c in range(n_cc):
                qs[ri % 3].dma_start(
                    x_sb[c][:, q4 * QT:(q4 + 1) * QT],
                    xT[c * P:(c + 1) * P, q4 * QT:(q4 + 1) * QT])
                ri += 1
            if q4 == 0:
                nc.scalar.dma_start(wv_sb[:], wv_s)
        for f in range(n_f):
            if f not in (0, n_jt):
                qs[ri % 3].dma_start(wqk_sb[f][:], wqk[f])
                ri += 1
        nc.scalar.dma_start(wp_sb[:], wp_s)

        nc.gpsimd.partition_broadcast(bv_bc[:], bv_row[:])
        nc.gpsimd.partition_broadcast(bp_bc[:], bp_row[:])
        nc.gpsimd.memset(ones_f[:], 1.0)
        nc.gpsimd.memset(tri[:], 1.0)
        nc.gpsimd.affine_select(
            out=tri[:], in_=tri[:], compare_op=mybir.AluOpType.is_ge,
            fill=0.0, base=0, channel_multiplier=-1, pattern=[[1, P]])

        # ---- AG segments ---------------------------------------------
        # pairs 0-2: two T-half segments. pair 3 (j-order [2,3,1,0]): one
        # T-half for j 2,3 plus two q-tile quarters so the final AG is small
        # and lands right after the shortest j.
        segs = {}   # (pr, j) -> [in_tile, out_tile, col_base, j_set]
        for p in range(n_pair):
            if p < n_pair - 1:
                for h in range(n_half):
                    ti = dram.tile([P, TH], BF16, tag=f"agi{p}_{h}",
                                   name=f"agi{p}_{h}")
                    to = dram.tile([2, P, TH], BF16, tag=f"ago{p}_{h}",
                                   name=f"ago{p}_{h}")
                    for j in (2 * h, 2 * h + 1):
                        segs[(p, j)] = [ti, to, (j % 2) * QT, {2 * h, 2 * h + 1}]
            else:
                ti = dram.tile([P, TH], BF16, tag=f"agi{p}_h1",
                               name=f"agi{p}_h1")
                to = dram.tile([2, P, TH], BF16, tag=f"ago{p}_h1",
                               name=f"ago{p}_h1")
                for j in (2, 3):
                    segs[(p, j)] = [ti, to, (j % 2) * QT, {2, 3}]
                for j in (0, 1):
                    ti = dram.tile([P, QT], BF16, tag=f"agi{p}_q{j}",
                                   name=f"agi{p}_q{j}")
                    to = dram.tile([2, P, QT], BF16, tag=f"ago{p}_q{j}",
                                   name=f"ago{p}_q{j}")
                    segs[(p, j)] = [ti, to, 0, {j}]

        # ---- compute atoms -------------------------------------------
        def v_atom(t):
            """V for t-chunk t: [128 t, CL] + bias, ones col per head."""
            pv = ps.tile([P, CL], F32, tag="qps", bufs=2, name="pv")
            for c in range(n_cc):
                nc.tensor.matmul(
                    pv[:], x_sb[c][:, t * KC:(t + 1) * KC], wv_sb[:, c, :],
                    start=(c == 0), stop=(c == n_cc - 1))
            nc.vector.tensor_copy(v_sb[t][:, :, HS:HS + 1], ones_f[:])
            nc.vector.tensor_add(
                v_sb[t][:, :, 0:HS],
                pv.rearrange("p (h e) -> p h e", e=HS),
                bv_bc.rearrange("p (h e) -> p h e", e=HS))

        def qk_atom(f, t):
            """q/k f-tile x one t-tile of 512: 8 matmuls + bias to SBUF."""
            pq = ps.tile([P, QT], F32, tag="qps", bufs=2, name="pq")
            for c in range(n_cc):
                nc.tensor.matmul(
                    pq[:], wqk_sb[f][:, c, :],
                    x_sb[c][:, t * QT:(t + 1) * QT],
                    start=(c == 0), stop=(c == n_cc - 1))
            nc.vector.tensor_scalar_add(
                qk_sb[f][:, t * QT:(t + 1) * QT], pq[:], bqk_sb[:, f:f + 1])

        proj_pend = []   # prefetched (p, t, [yt0, yt1]) awaiting matmul

        def proj_fetch(p, t):
            j = t // KPQ
            _, to, col_base, _ = segs[(p, j)]
            col = col_base + (t % KPQ) * P
            yts = []
            for gp in range(2):
                yt = att.tile([P, P], BF16, tag="yt", bufs=12, name="yt")
                nc.sync.dma_start(yt[:], to[gp, :, col:col + P])
                yts.append(yt)
            proj_pend.append((p, t, yts))

        def proj_exec(p, t, yts):
            po = ps.tile([P, CL], F32, tag="qps", bufs=2, name="po")
            for gp in range(2):
                nc.tensor.matmul(po[:], yts[gp][:],
                                 wp_sb[:, gp * n_pair + p, :],
                                 start=(gp == 0), stop=(gp == 1))
            if p == 0:
                nc.vector.tensor_add(oacc[t][:], po[:], bp_bc[:])
            else:
                nc.vector.tensor_add(oacc[t][:], oacc[t][:], po[:])
            if p == n_pair - 1:
                nc.sync.dma_start(out_ext[t * P:(t + 1) * P, :], oacc[t][:])

        def proj_atom(p, t):
            """Projection tile with 3-deep DMA prefetch."""
            proj_fetch(p, t)
            if len(proj_pend) > 3:
                proj_exec(*proj_pend.pop(0))

        def proj_drain():
            while proj_pend:
                proj_exec(*proj_pend.pop(0))

        # ---- filler queue --------------------------------------------
        # (min_pair, min_j, thunk): atom may only be emitted at or after
        # attention position (min_pair, min_j) - proj needs its AG landed,
        # pair-0 q/k and V atoms need their x quarter (position-gated so the
        # attention can start right after x quarter 0 lands).
        filler = []
        filler.append((0, 1, lambda: qk_atom(0, 1)))
        filler.append((0, 1, lambda: qk_atom(n_jt, 1)))
        for t in range(4, 8):
            filler.append((0, 1, (lambda t=t: v_atom(t))))
        for q4 in (2, 3):
            filler.append((0, q4, (lambda t=q4: qk_atom(0, t))))
            filler.append((0, q4, (lambda t=q4: qk_atom(n_jt, t))))
            for t in range(4 * q4, 4 * q4 + 4):
                filler.append((0, q4, (lambda t=t: v_atom(t))))
        for pr in range(1, n_pair):
            for t in range(n_jt):
                filler.append((0, 0, (lambda f=n_jt + pr, t=t: qk_atom(f, t))))
                filler.append((0, 0, (lambda f=pr, t=t: qk_atom(f, t))))
        for p in range(n_pair):
            # positions are (pair, emission-idx). Pairs 0-2: half AGs issue
            # after idx 1 and 3. Pair 3 (j-order [2,3,1,0]): h1 AG at idx 1,
            # quarter AGs after idx 2 and 3 -> those drain in the tail,
            # ready-first (t order 8..15, 4..7, 0..3).
            if p < n_pair - 1:
                ts = list(range(T // P))
            else:
                ts = list(range(8, 16)) + list(range(4, 8)) + list(range(4))
            for t in ts:
                h = t // (TH // P)
                if p < n_pair - 1:
                    mp, mj = (p + 1, 0) if h == 0 else (p + 1, 2)
                else:
                    mp, mj = (p, 2) if h == 1 else (p, n_jt)
                filler.append((mp, mj, (lambda p=p, t=t: proj_atom(p, t))))

        def pop_filler(pr, j):
            for idx, (mp, mj, thunk) in enumerate(filler):
                if (mp, mj) <= (pr, j):
                    filler.pop(idx)
                    thunk()
                    return True
            return False

        # ---- attention -----------------------------------------------
        def attention_pair(pr, js):
            """Both heads of pair pr; scores^T [k, q] stripes, 2-deep
            pipelined st -> exp -> av so PE never waits on ACT."""
            kT = qk_sb[n_pair + pr]
            qTt = qk_sb[pr]
            done_j = set()
            ag_done = set()
            for idx, j in enumerate(js):
                yps = {rr: ps.tile([P, QT], F32, tag=f"yp{rr}", bufs=1,
                                   name=f"yp{rr}") for rr in range(2)}
                imax = KPQ * j + KPQ
                # fillers here keep the PE busy while the previous j's
                # normalize chain releases the yp PSUM banks
                pop_filler(pr, idx)
                pop_filler(pr, idx)
                pend = []   # pipelined (i, off, pt) awaiting av

                def av(iv, offv, ptv):
                    for rr in range(2):
                        nc.tensor.matmul(
                            yps[rr][0:HS + 1, offv:QT],
                            v_sb[iv][:, 2 * pr + rr, 0:HS + 1],
                            ptv[:, rr, offv:QT],
                            start=(iv == 0), stop=(iv == imax - 1))

                for i in range(imax):
                    diag = (i // KPQ == j)
                    # causally trim diagonal chunks to q >= i*KC
                    off = KC * (i % KPQ) if diag else 0
                    st = ps.tile([P, 2, QT], F32, tag="st", bufs=2,
                                 name="st")
                    for rr in range(2):
                        ro = HS * rr
                        nc.tensor.matmul(
                            st[:, rr, off:QT],
                            kT[ro:ro + HS, i * KC:(i + 1) * KC],
                            qTt[ro:ro + HS, j * QT + off:(j + 1) * QT],
                            start=True, stop=True)
                    pt = att.tile([P, 2, QT], BF16, tag="pt", bufs=4,
                                  name="pt")
                    nc.scalar.activation(
                        pt[:, :, off:QT], st[:, :, off:QT],
                        mybir.ActivationFunctionType.Exp, scale=scale)
                    if diag:
                        for rr in range(2):
                            # zero above the diagonal in the leading
                            # 128x128 triangle, in place
                            nc.vector.tensor_mul(
                                pt[:, rr, off:off + KC],
                                pt[:, rr, off:off + KC], tri[:])
                    pend.append((i, off, pt))
                    if len(pend) > 2:
                        av(*pend.pop(0))
                    if i % 2 == 1:
                        pop_filler(pr, idx)
                while pend:
                    av(*pend.pop(0))
                for rr in range(2):
                    ro = HS * rr
                    # ONE full-bank copy frees the yp PSUM bank immediately
                    # (the next j's first att.v otherwise stalls on the whole
                    # normalize chain). The recip ucode misreads inputs at a
                    # nonzero base partition (silent corruption on HW), so
                    # stage the sum row into a base-0 tile first; it must
                    # also not read PSUM directly.
                    stg = att.tile([P, QT], F32, tag="stg", bufs=3,
                                   name="stg")
                    nc.vector.tensor_copy(stg[:], yps[rr][:])
                    row = att.tile([1, QT], F32, tag="row", bufs=3,
                                   name="row")
                    nc.vector.tensor_copy(row[:], stg[HS:HS + 1, :])
                    rec = att.tile([1, QT], F32, tag="rec", bufs=3,
                                   name="rec")
                    nc.vector.reciprocal_approx_fast(rec[:], row[:])
                    rb = att.tile([HS, QT], F32, tag="rb", bufs=3, name="rb")
                    nc.gpsimd.partition_broadcast(rb[:], rec[:])
                    yn = att.tile([HS, QT], BF16, tag="yn", bufs=4,
                                  name="yn")
                    nc.vector.tensor_mul(yn[:], stg[0:HS, :], rb[:])
                    ti, _, col_base, _ = segs[(pr, j)]
                    nc.sync.dma_start(
                        ti[ro:ro + HS, col_base:col_base + QT], yn[:])
                done_j.add(j)
                ti, to, _, j_set = segs[(pr, j)]
                if j_set <= done_j and id(ti) not in ag_done:
                    ag_done.add(id(ti))
                    nc.gpsimd.collective_compute(
                        "AllGather", mybir.AluOpType.bypass,
                        replica_groups=PAIRS,
                        ins=[ti.opt()], outs=[to.opt()])

        # ---- schedule ------------------------------------------------
        # minimal upfront: just what pair-0 j=0 needs (x quarter 0 derived);
        # everything else flows in through the filler queue
        qk_atom(0, 0)
        qk_atom(n_jt, 0)
        for t in range(4):
            v_atom(t)

        for pr in range(n_pair):
            js = [2, 3, 1, 0] if pr == n_pair - 1 else list(range(n_jt))
            attention_pair(pr, js)
        # drain remaining fillers (last AG half's projection tiles)
        while pop_filler(n_pair - 1, n_jt):
            pass
        proj_drain()

    nc.compile()
    return nc


def shard_inputs(x, w_attn, b_attn, w_proj, b_proj):
    """Slice/transpose/shuffle full inputs into 8 per-core input maps."""
    Bq, T, C = x.shape
    CL = C // 2
    n_cc = C // P
    n_f = 2 * CL // P
    bf = ml_dtypes.bfloat16
    in_maps = []
    for i in range(N_CORES):
        b, g = i // 2, i % 2
        sl = slice(CL * g, CL * (g + 1))
        wq = w_attn[:, sl]
        wk = w_attn[:, C + CL * g:C + CL * (g + 1)]
        wvv = w_attn[:, 2 * C + CL * g:2 * C + CL * (g + 1)]
        wqk = np.concatenate([wq, wk], axis=1)          # [C, 2CL]
        # [C, 2CL] -> [f, p, c, m]: row r = c*128+p, col = f*128+m
        wqk_s = np.ascontiguousarray(
            wqk.reshape(n_cc, P, n_f, P).transpose(2, 1, 0, 3)).astype(bf)
        wv_shuf = np.ascontiguousarray(
            wvv.reshape(n_cc, P, CL).transpose(1, 0, 2)).astype(bf)
        wp_shuf = np.ascontiguousarray(
            w_proj[:, sl].reshape(n_cc, P, CL).transpose(1, 0, 2)).astype(bf)
        in_maps.append({
            "xT": np.ascontiguousarray(x[b].T).astype(bf),
            "wqk": wqk_s,
            "wv_s": wv_shuf,
            "wp_s": wp_shuf,
            "bqk": np.ascontiguousarray(
                np.concatenate([b_attn[sl],
                                b_attn[C + CL * g:C + CL * (g + 1)]])
                .reshape(n_f, P).T),
            "bv": np.ascontiguousarray(b_attn[2 * C + CL * g:2 * C + CL * (g + 1)]),
            "bp": np.ascontiguousarray(b_proj[sl]),
        })
    return in_maps


def gather_outputs(results, B, T, C):
    CL = C // 2
    out = np.empty((B, T, C), dtype=np.float32)
    for i in range(N_CORES):
        b, g = i // 2, i % 2
        out[b, :, CL * g:CL * (g + 1)] = results[i]["out"]
    return out


_NC_CACHE = {}


def get_nc(T, C):
    key = (T, C)
    if key not in _NC_CACHE:
        _NC_CACHE[key] = build_nc(T=T, C=C, HL=C // HS // 2)
    return _NC_CACHE[key]


def kernel(x, w_attn, b_attn, w_proj, b_proj):
    x = np.asarray(x, dtype=np.float32)
    w_attn = np.asarray(w_attn, dtype=np.float32)
    b_attn = np.asarray(b_attn, dtype=np.float32)
    w_proj = np.asarray(w_proj, dtype=np.float32)
    b_proj = np.asarray(b_proj, dtype=np.float32)

    Bq, T, C = x.shape
    nc = get_nc(T, C)

    in_maps = shard_inputs(x, w_attn, b_attn, w_proj, b_proj)
    trace = os.environ.get("KERNEL_TRACE", "0") == "1"
    res = bass_utils.run_bass_kernel_spmd(
        nc, in_maps, core_ids=list(range(N_CORES)), trace=trace)
    if trace and res.exec_time_ns is not None:
        print(f"HW exec time: {res.exec_time_ns} ns", flush=True)
        kernel.last_exec_time_ns = res.exec_time_ns
        kernel.last_results = res
    return gather_outputs(res.results, Bq, T, C)


# revision 63
# speedup vs baseline: 1.0088x; 1.0088x over previous
"""Causal self-attention (B=4, T=2048, C=1024, H=16) on 8 TRN2 NeuronCores.

Sharding: data-parallel on batch (4) x tensor-parallel on heads (2 groups of
8). Core i handles batch i//2 and head-group i%2. Per core:
  - QKV matmuls for its head-group's weight columns. q,k are produced in
    transposed [feature, T] layout; v in natural [T, feature] layout with a
    ones column per head (sum(exp) accumulates in the attention matmul).
  - Causal attention per head-pair in scores^T layout [k, q]. No max
    subtraction: scores*hs^-0.5 are O(+-10), exp is safe. Fully-masked
    k-blocks are skipped; diagonal blocks are N-trimmed to the causal q-range
    and the remaining 128x128 triangle is masked with a DVE multiply against
    a precomputed 0/1 mask (NOT gpsimd affine_select - that serialized the
    collective queue and stalled the tail at half HAM clock).
  - The score->exp->att.v chain is software-pipelined 2 deep: PE emits
    st(i), ACT exp(i), PE av(i-2), so the PE never waits on the ACT engine.
    Filler work (V chunks, later pairs' q/k tiles, projection tiles) is
    injected every other iteration to keep the PE dense (HAM clock warm).
  - y^T is exchanged between the two cores of a batch with pairwise
    AllGathers per (pair, T-half); the last pair runs j-order [2,3,1,0] and
    splits its trailing half into two q-tile quarter AGs, so the final
    collective is small and lands right after the shortest j.
  - Projection accumulates in SBUF as AG chunks arrive (position-gated
    filler atoms with 3-deep yt DMA prefetch); b_proj folded in.
Host shuffles weights/biases into [p, ...]-contiguous layouts so every DMA
moves 1-8KB lines per partition (a bqk rearrange was a 1024-descriptor bomb);
x streams in T-quarter order so the first matmuls start after ~2.5MB.

dtypes: all matmul operands bf16; every accumulation fp32 in PSUM; softmax
normalization fp32 (measured ~5e-3 fro vs fp32 reference; gate is 2e-2).

Measured on HW: 345.6us exec (twice, +-15ns), vs 426-488us for the prior
session's baseline. PE active ~266us (~225us real work at 2.4GHz - the
scores/att.v matmuls are inherently ~50% PE-utilized at K=64/M=65);
remaining overheads: ~25us DMA-ring-bound startup, ~20us AG-bound tail,
HAM half-clock stretch around idle windows.

HW gotchas (CoreSim passes all of these; only real HW fails):
  - a single 65-partition DVE copy from PSUM silently corrupts data
  - gpsimd custom-DVE reciprocal misreads PSUM (stage rows in SBUF first)
  - junk "heater" matmuls and fine-grained (16x) AllGathers both regress
"""

import os
import sys
from contextlib import ExitStack

import numpy as np
import ml_dtypes

if "/opt/trn_rl_repo" not in sys.path:
    sys.path.insert(0, "/opt/trn_rl_repo")

import concourse.bass as bass
import concourse.mybir as mybir
import concourse.tile as tile
from concourse import bacc
from concourse import bass_utils

F32 = mybir.dt.float32
BF16 = mybir.dt.bfloat16
P = 128          # SBUF partitions
QT = 512         # q tile (matmul free dim)
KC = 128         # k chunk (psum partition dim)
HS = 64          # head size
KPQ = QT // KC   # k chunks per q tile

N_CORES = 8
PAIRS = [[0, 1], [2, 3], [4, 5], [6, 7]]

B_FULL, T_FULL, C_FULL, H_FULL = 4, 2048, 1024, 16


def build_nc(T=T_FULL, C=C_FULL, HL=H_FULL // 2):
    """Build the SPMD graph for one core (all 8 cores run the same graph).

    Per-core input tensors:
      xT    [C, T] bf16       x[b] transposed
      wqk   [2CL/P, P, C/P, P] bf16  w_attn q|k cols, host-shuffled [f,p,c,m]
      wv_s  [P, C/P, CL] bf16 w_attn v cols, host-shuffled [p,c,m]
      wp_s  [P, C/P, CL] bf16 w_proj cols for this core's output half
      bqk   [2*CL] f32, bv [CL] f32, bp [CL] f32
    Output: out [T, CL] f32.
    """
    CL = HL * HS                 # local width (q, k, v, out-cols each)
    n_cc = C // P                # x feature chunks (8)
    n_f = 2 * CL // P            # q|k f-tiles (4 q then 4 k)
    n_jt = T // QT               # q tiles (4)
    n_kt = T // KC               # k chunks / v t-chunks (16)
    n_pair = HL // 2             # head pairs (4)
    n_half = 2                   # T halves for AG chunking
    TH = T // n_half
    scale = HS ** -0.5

    nc = bacc.Bacc("TRN2", target_bir_lowering=False, debug=False,
                   num_devices=N_CORES)

    xT = nc.dram_tensor("xT", [C, T], BF16, kind="ExternalInput").ap()
    wqk = nc.dram_tensor("wqk", [n_f, P, n_cc, P], BF16,
                         kind="ExternalInput").ap()
    wv_s = nc.dram_tensor("wv_s", [P, n_cc, CL], BF16,
                          kind="ExternalInput").ap()
    wp_s = nc.dram_tensor("wp_s", [P, n_cc, CL], BF16,
                          kind="ExternalInput").ap()
    bqk = nc.dram_tensor("bqk", [P, 2 * CL // P], F32,
                         kind="ExternalInput").ap()
    bv = nc.dram_tensor("bv", [CL], F32, kind="ExternalInput").ap()
    bp = nc.dram_tensor("bp", [CL], F32, kind="ExternalInput").ap()
    out_ext = nc.dram_tensor("out", [T, CL], F32, kind="ExternalOutput").ap()

    with ExitStack() as ctx:
        tc = ctx.enter_context(tile.TileContext(nc))

        persist = ctx.enter_context(tc.tile_pool(name="persist", bufs=1))
        dram = ctx.enter_context(tc.tile_pool(name="dram", bufs=1, space="DRAM"))
        # st 2x2 banks + yp0 + yp1 + qps 2 = 8 banks
        ps = ctx.enter_context(tc.tile_pool(name="ps", bufs=1, space="PSUM"))
        att = ctx.enter_context(tc.tile_pool(name="att", bufs=1))

        # ---- persistent SBUF tiles -----------------------------------
        wqk_sb = [persist.tile([P, n_cc, P], BF16, tag=f"wqk{f}",
                               name=f"wqk{f}") for f in range(n_f)]
        wv_sb = persist.tile([P, n_cc, CL], BF16, tag="wv", name="wv")
        wp_sb = persist.tile([P, n_cc, CL], BF16, tag="wp", name="wp")
        x_sb = [persist.tile([P, T], BF16, tag=f"x{c}", name=f"x{c}")
                for c in range(n_cc)]
        qk_sb = [persist.tile([P, T], BF16, tag=f"qk{f}", name=f"qk{f}")
                 for f in range(n_f)]
        v_sb = [persist.tile([P, HL, HS + 2], BF16, tag=f"v{t}",
                             name=f"v{t}") for t in range(n_kt)]
        oacc = [persist.tile([P, CL], F32, tag=f"oacc{t}", name=f"oacc{t}")
                for t in range(T // P)]
        bqk_sb = persist.tile([P, n_f], F32, tag="bqk", name="bqk_sb")
        bv_bc = persist.tile([P, CL], F32, tag="bv_bc", name="bv_bc")
        bp_bc = persist.tile([P, CL], F32, tag="bp_bc", name="bp_bc")
        ones_f = persist.tile([P, HL, 1], F32, tag="ones_f", name="ones_f")
        # tri[p, g] = 1 where g >= p else 0 (keep-at-or-above-diagonal)
        tri = persist.tile([P, P], BF16, tag="tri", name="tri")

        # ---- input DMAs, in consumption order ------------------------
        # the three DGE rings (sync/gpsimd/scalar) give ~130GB/s aggregate;
        # order transfers so compute can start incrementally: weights for the
        # first atoms, then x by T-quarter so qk/V atoms unlock quarter by
        # quarter
        qs = [nc.sync, nc.gpsimd, nc.scalar]
        # tiny bias rows first (the first qk bias-add gates attention start;
        # bqk is host-shuffled to [p, f] so this is one descriptor per
        # partition, not a 1024-descriptor gather)
        nc.sync.dma_start(bqk_sb[:], bqk)
        bv_row = att.tile([1, CL], F32, tag="brow", bufs=2, name="bv_row")
        nc.sync.dma_start(bv_row[:], bv.rearrange("(o c) -> o c", o=1))
        bp_row = att.tile([1, CL], F32, tag="brow", bufs=2, name="bp_row")
        nc.sync.dma_start(bp_row[:], bp.rearrange("(o c) -> o c", o=1))
        nc.sync.dma_start(wqk_sb[0][:], wqk[0])
        nc.gpsimd.dma_start(wqk_sb[n_jt][:], wqk[n_jt])
        ri = 0
        for q4 in range(4):
            for c in range(n_cc):
                qs[ri % 3].dma_start(
                    x_sb[c][:, q4 * QT:(q4 + 1) * QT],
                    xT[c * P:(c + 1) * P, q4 * QT:(q4 + 1) * QT])
                ri += 1
            if q4 == 0:
                nc.scalar.dma_start(wv_sb[:], wv_s)
        for f in range(n_f):
            if f not in (0, n_jt):
                qs[ri % 3].dma_start(wqk_sb[f][:], wqk[f])
                ri += 1
        nc.scalar.dma_start(wp_sb[:], wp_s)

        nc.gpsimd.partition_broadcast(bv_bc[:], bv_row[:])
        nc.gpsimd.partition_broadcast(bp_bc[:], bp_row[:])
        nc.gpsimd.memset(ones_f[:], 1.0)
        nc.gpsimd.memset(tri[:], 1.0)
        nc.gpsimd.affine_select(
            out=tri[:], in_=tri[:], compare_op=mybir.AluOpType.is_ge,
            fill=0.0, base=0, channel_multiplier=-1, pattern=[[1, P]])

        # ---- AG segments ---------------------------------------------
        # pairs 0-2: two T-half segments. pair 3 (j-order [2,3,1,0]): one
        # T-half for j 2,3 plus two q-tile quarters so the final AG is small
        # and lands right after the shortest j.
        segs = {}   # (pr, j) -> [in_tile, out_tile, col_base, j_set]
        for p in range(n_pair):
            if p < n_pair - 1:
                for h in range(n_half):
                    ti = dram.tile([P, TH], BF16, tag=f"agi{p}_{h}",
                                   name=f"agi{p}_{h}")
                    to = dram.tile([2, P, TH], BF16, tag=f"ago{p}_{h}",
                                   name=f"ago{p}_{h}")
                    for j in (2 * h, 2 * h + 1):
                        segs[(p, j)] = [ti, to, (j % 2) * QT, {2 * h, 2 * h + 1}]
            else:
                ti = dram.tile([P, TH], BF16, tag=f"agi{p}_h1",
                               name=f"agi{p}_h1")
                to = dram.tile([2, P, TH], BF16, tag=f"ago{p}_h1",
                               name=f"ago{p}_h1")
                for j in (2, 3):
                    segs[(p, j)] = [ti, to, (j % 2) * QT, {2, 3}]
                for j in (0, 1):
                    ti = dram.tile([P, QT], BF16, tag=f"agi{p}_q{j}",
                                   name=f"agi{p}_q{j}")
                    to = dram.tile([2, P, QT], BF16, tag=f"ago{p}_q{j}",
                                   name=f"ago{p}_q{j}")
                    segs[(p, j)] = [ti, to, 0, {j}]

        # ---- compute atoms -------------------------------------------
        def v_atom(t):
            """V for t-chunk t: [128 t, CL] + bias, ones col per head."""
            pv = ps.tile([P, CL], F32, tag="qps", bufs=2, name="pv")
            for c in range(n_cc):
                nc.tensor.matmul(
                    pv[:], x_sb[c][:, t * KC:(t + 1) * KC], wv_sb[:, c, :],
                    start=(c == 0), stop=(c == n_cc - 1))
            nc.vector.tensor_copy(v_sb[t][:, :, HS:HS + 1], ones_f[:])
            nc.vector.tensor_add(
                v_sb[t][:, :, 0:HS],
                pv.rearrange("p (h e) -> p h e", e=HS),
                bv_bc.rearrange("p (h e) -> p h e", e=HS))

        def qk_atom(f, t):
            """q/k f-tile x one t-tile of 512: 8 matmuls + bias to SBUF."""
            pq = ps.tile([P, QT], F32, tag="qps", bufs=2, name="pq")
            for c in range(n_cc):
                nc.tensor.matmul(
                    pq[:], wqk_sb[f][:, c, :],
                    x_sb[c][:, t * QT:(t + 1) * QT],
                    start=(c == 0), stop=(c == n_cc - 1))
            nc.vector.tensor_scalar_add(
                qk_sb[f][:, t * QT:(t + 1) * QT], pq[:], bqk_sb[:, f:f + 1])

        proj_pend = []   # prefetched (p, t, [yt0, yt1]) awaiting matmul

        def proj_fetch(p, t):
            j = t // KPQ
            _, to, col_base, _ = segs[(p, j)]
            col = col_base + (t % KPQ) * P
            yts = []
            for gp in range(2):
                yt = att.tile([P, P], BF16, tag="yt", bufs=12, name="yt")
                nc.sync.dma_start(yt[:], to[gp, :, col:col + P])
                yts.append(yt)
            proj_pend.append((p, t, yts))

        def proj_exec(p, t, yts):
            po = ps.tile([P, CL], F32, tag="qps", bufs=2, name="po")
            for gp in range(2):
                nc.tensor.matmul(po[:], yts[gp][:],
                                 wp_sb[:, gp * n_pair + p, :],
                                 start=(gp == 0), stop=(gp == 1))
            if p == 0:
                nc.vector.tensor_add(oacc[t][:], po[:], bp_bc[:])
            else:
                nc.vector.tensor_add(oacc[t][:], oacc[t][:], po[:])
            if p == n_pair - 1:
                nc.sync.dma_start(out_ext[t * P:(t + 1) * P, :], oacc[t][:])

        def proj_atom(p, t):
            """Projection tile with 3-deep DMA prefetch."""
            proj_fetch(p, t)
            if len(proj_pend) > 3:
                proj_exec(*proj_pend.pop(0))

        def proj_drain():
            while proj_pend:
                proj_exec(*proj_pend.pop(0))

        # ---- filler queue --------------------------------------------
        # (min_pair, min_j, thunk): atom may only be emitted at or after
        # attention position (min_pair, min_j) - proj needs its AG landed,
        # pair-0 q/k and V atoms need their x quarter (position-gated so the
        # attention can start right after x quarter 0 lands).
        filler = []
        filler.append((0, 1, lambda: qk_atom(0, 1)))
        filler.append((0, 1, lambda: qk_atom(n_jt, 1)))
        for t in range(4, 8):
            filler.append((0, 1, (lambda t=t: v_atom(t))))
        for q4 in (2, 3):
            filler.append((0, q4, (lambda t=q4: qk_atom(0, t))))
            filler.append((0, q4, (lambda t=q4: qk_atom(n_jt, t))))
            for t in range(4 * q4, 4 * q4 + 4):
                filler.append((0, q4, (lambda t=t: v_atom(t))))
        for pr in range(1, n_pair):
            for t in range(n_jt):
                filler.append((0, 0, (lambda f=n_jt + pr, t=t: qk_atom(f, t))))
                filler.append((0, 0, (lambda f=pr, t=t: qk_atom(f, t))))
        for p in range(n_pair):
            # positions are (pair, emission-idx). Pairs 0-2: half AGs issue
            # after idx 1 and 3. Pair 3 (j-order [2,3,1,0]): h1 AG at idx 1,
            # quarter AGs after idx 2 and 3 -> those drain in the tail,
            # ready-first (t order 8..15, 4..7, 0..3).
            if p < n_pair - 1:
                ts = list(range(T // P))
            else:
                ts = list(range(8, 16)) + list(range(4, 8)) + list(range(4))
            for t in ts:
                h = t // (TH // P)
                if p < n_pair - 1:
                    mp, mj = (p + 1, 0) if h == 0 else (p + 1, 2)
                else:
                    mp, mj = (p, 2) if h == 1 else (p, n_jt)
                filler.append((mp, mj, (lambda p=p, t=t: proj_atom(p, t))))

        def pop_filler(pr, j):
            for idx, (mp, mj, thunk) in enumerate(filler):
                if (mp, mj) <= (pr, j):
                    filler.pop(idx)
                    thunk()
                    return True
            return False

        # ---- attention -----------------------------------------------
        def attention_pair(pr, js):
            """Both heads of pair pr; scores^T [k, q] stripes, 2-deep
            pipelined st -> exp -> av so PE never waits on ACT."""
            kT = qk_sb[n_pair + pr]
            qTt = qk_sb[pr]
            done_j = set()
            ag_done = set()
            for idx, j in enumerate(js):
                yps = {rr: ps.tile([P, QT], F32, tag=f"yp{rr}", bufs=1,
                                   name=f"yp{rr}") for rr in range(2)}
                imax = KPQ * j + KPQ
                # fillers here keep the PE busy while the previous j's
                # normalize chain releases the yp PSUM banks
                pop_filler(pr, idx)
                pop_filler(pr, idx)
                pend = []   # pipelined (i, off, pt) awaiting av

                def av(iv, offv, ptv):
                    for rr in range(2):
                        nc.tensor.matmul(
                            yps[rr][0:HS + 1, offv:QT],
                            v_sb[iv][:, 2 * pr + rr, 0:HS + 1],
                            ptv[:, rr, offv:QT],
                            start=(iv == 0), stop=(iv == imax - 1))

                for i in range(imax):
                    diag = (i // KPQ == j)
                    # causally trim diagonal chunks to q >= i*KC
                    off = KC * (i % KPQ) if diag else 0
                    st = ps.tile([P, 2, QT], F32, tag="st", bufs=2,
                                 name="st")
                    for rr in range(2):
                        ro = HS * rr
                        nc.tensor.matmul(
                            st[:, rr, off:QT],
                            kT[ro:ro + HS, i * KC:(i + 1) * KC],
                            qTt[ro:ro + HS, j * QT + off:(j + 1) * QT],
                            start=True, stop=True)
                    pt = att.tile([P, 2, QT], BF16, tag="pt", bufs=4,
                                  name="pt")
                    nc.scalar.activation(
                        pt[:, :, off:QT], st[:, :, off:QT],
                        mybir.ActivationFunctionType.Exp, scale=scale)
                    if diag:
                        for rr in range(2):
                            # zero above the diagonal in the leading
                            # 128x128 triangle, in place
                            nc.vector.tensor_mul(
                                pt[:, rr, off:off + KC],
                                pt[:, rr, off:off + KC], tri[:])
                    pend.append((i, off, pt))
                    if len(pend) > 2:
                        av(*pend.pop(0))
                    if i % 2 == 1:
                        pop_filler(pr, idx)
                while pend:
                    av(*pend.pop(0))
                for rr in range(2):
                    ro = HS * rr
                    # ONE full-bank copy frees the yp PSUM bank immediately
                    # (the next j's first att.v otherwise stalls on the whole
                    # normalize chain, ~2us/j of PE idle). The reciprocal
                    # must be the NATIVE InstReciprocal: the custom-DVE
                    # approx ucode silently corrupts on nonzero-base-
                    # partition inputs like stg[64:65], and adding a base-0
                    # staging copy (5-op chain) collapses the clock 2x.
                    stg = att.tile([P, QT], F32, tag="stg", bufs=3,
                                   name="stg")
                    nc.vector.tensor_copy(stg[:], yps[rr][:])
                    rec = att.tile([1, QT], F32, tag="rec", bufs=3,
                                   name="rec")
                    nc.vector.reciprocal(rec[:], stg[HS:HS + 1, :])
                    rb = att.tile([HS, QT], F32, tag="rb", bufs=3, name="rb")
                    nc.gpsimd.partition_broadcast(rb[:], rec[:])
                    yn = att.tile([HS, QT], BF16, tag="yn", bufs=4,
                                  name="yn")
                    nc.vector.tensor_mul(yn[:], stg[0:HS, :], rb[:])
                    ti, _, col_base, _ = segs[(pr, j)]
                    nc.sync.dma_start(
                        ti[ro:ro + HS, col_base:col_base + QT], yn[:])
                done_j.add(j)
                ti, to, _, j_set = segs[(pr, j)]
                if j_set <= done_j and id(ti) not in ag_done:
                    ag_done.add(id(ti))
                    nc.gpsimd.collective_compute(
                        "AllGather", mybir.AluOpType.bypass,
                        replica_groups=PAIRS,
                        ins=[ti.opt()], outs=[to.opt()])

        # ---- schedule ------------------------------------------------
        # minimal upfront: just what pair-0 j=0 needs (x quarter 0 derived);
        # everything else flows in through the filler queue
        qk_atom(0, 0)
        qk_atom(n_jt, 0)
        for t in range(4):
            v_atom(t)

        for pr in range(n_pair):
            js = [2, 3, 1, 0] if pr == n_pair - 1 else list(range(n_jt))
            attention_pair(pr, js)
        # drain remaining fillers (last AG half's projection tiles)
        while pop_filler(n_pair - 1, n_jt):
            pass
        proj_drain()

    nc.compile()
    return nc


def shard_inputs(x, w_attn, b_attn, w_proj, b_proj):
    """Slice/transpose/shuffle full inputs into 8 per-core input maps."""
    Bq, T, C = x.shape
    CL = C // 2
    n_cc = C // P
    n_f = 2 * CL // P
    bf = ml_dtypes.bfloat16
    in_maps = []
    for i in range(N_CORES):
        b, g = i // 2, i % 2
        sl = slice(CL * g, CL * (g + 1))
        wq = w_attn[:, sl]
        wk = w_attn[:, C + CL * g:C + CL * (g + 1)]
        wvv = w_attn[:, 2 * C + CL * g:2 * C + CL * (g + 1)]
        wqk = np.concatenate([wq, wk], axis=1)          # [C, 2CL]
        # [C, 2CL] -> [f, p, c, m]: row r = c*128+p, col = f*128+m
        wqk_s = np.ascontiguousarray(
            wqk.reshape(n_cc, P, n_f, P).transpose(2, 1, 0, 3)).astype(bf)
        wv_shuf = np.ascontiguousarray(
            wvv.reshape(n_cc, P, CL).transpose(1, 0, 2)).astype(bf)
        wp_shuf = np.ascontiguousarray(
            w_proj[:, sl].reshape(n_cc, P, CL).transpose(1, 0, 2)).astype(bf)
        in_maps.append({
            "xT": np.ascontiguousarray(x[b].T).astype(bf),
            "wqk": wqk_s,
            "wv_s": wv_shuf,
            "wp_s": wp_shuf,
            "bqk": np.ascontiguousarray(
                np.concatenate([b_attn[sl],
                                b_attn[C + CL * g:C + CL * (g + 1)]])
                .reshape(n_f, P).T),
            "bv": np.ascontiguousarray(b_attn[2 * C + CL * g:2 * C + CL * (g + 1)]),
            "bp": np.ascontiguousarray(b_proj[sl]),
        })
    return in_maps


def gather_outputs(results, B, T, C):
    CL = C // 2
    out = np.empty((B, T, C), dtype=np.float32)
    for i in range(N_CORES):
        b, g = i // 2, i % 2
        out[b, :, CL * g:CL * (g + 1)] = results[i]["out"]
    return out


_NC_CACHE = {}


def get_nc(T, C):
    key = (T, C)
    if key not in _NC_CACHE:
        _NC_CACHE[key] = build_nc(T=T, C=C, HL=C // HS // 2)
    return _NC_CACHE[key]


def kernel(x, w_attn, b_attn, w_proj, b_proj):
    x = np.asarray(x, dtype=np.float32)
    w_attn = np.asarray(w_attn, dtype=np.float32)
    b_attn = np.asarray(b_attn, dtype=np.float32)
    w_proj = np.asarray(w_proj, dtype=np.float32)
    b_proj = np.asarray(b_proj, dtype=np.float32)

    Bq, T, C = x.shape
    nc = get_nc(T, C)

    in_maps = shard_inputs(x, w_attn, b_attn, w_proj, b_proj)
    trace = os.environ.get("KERNEL_TRACE", "0") == "1"
    res = bass_utils.run_bass_kernel_spmd(
        nc, in_maps, core_ids=list(range(N_CORES)), trace=trace)
    if trace and res.exec_time_ns is not None:
        print(f"HW exec time: {res.exec_time_ns} ns", flush=True)
        kernel.last_exec_time_ns = res.exec_time_ns
        kernel.last_results = res
    return gather_outputs(res.results, Bq, T, C)


# revision 67
# speedup vs baseline: 1.1137x; 1.1039x over previous
"""Causal self-attention (B=4, T=2048, C=1024, H=16) on 8 TRN2 NeuronCores.

Sharding: data-parallel on batch (4) x tensor-parallel on heads (2 groups of
8). Core i handles batch i//2 and head-group i%2. Per core:
  - QKV matmuls for its head-group's weight columns. q,k are produced in
    transposed [feature, T] layout; v in natural [T, feature] layout with a
    ones column per head (sum(exp) accumulates in the attention matmul).
  - Causal attention per head-pair in scores^T layout [k, q]. No max
    subtraction: scores*hs^-0.5 are O(+-10), exp is safe. Fully-masked
    k-blocks are skipped; diagonal blocks are N-trimmed to the causal q-range
    and the remaining 128x128 triangle is masked with a DVE multiply against
    a precomputed 0/1 mask (NOT gpsimd affine_select - that serialized the
    collective queue and stalled the tail at half HAM clock).
  - The score->exp->att.v chain is software-pipelined 2 deep: PE emits
    st(i), ACT exp(i), PE av(i-2), so the PE never waits on the ACT engine.
    Filler work (V chunks, later pairs' q/k tiles, projection tiles) is
    injected every other iteration to keep the PE dense (HAM clock warm).
  - y^T is exchanged between the two cores of a batch with pairwise
    AllGathers per (pair, T-half); the last pair runs j-order [2,3,1,0] and
    splits its trailing half into two q-tile quarter AGs, so the final
    collective is small and lands right after the shortest j.
  - Projection accumulates in SBUF as AG chunks arrive (position-gated
    filler atoms with 3-deep yt DMA prefetch); b_proj folded in.
Host shuffles weights/biases into [p, ...]-contiguous layouts so every DMA
moves 1-8KB lines per partition (a bqk rearrange was a 1024-descriptor bomb);
x streams in T-quarter order so the first matmuls start after ~2.5MB.

dtypes: all matmul operands bf16; every accumulation fp32 in PSUM; softmax
normalization fp32 (measured ~5e-3 fro vs fp32 reference; gate is 2e-2).

Measured on HW: 345.6us exec (twice, +-15ns), vs 426-488us for the prior
session's baseline. PE active ~266us (~225us real work at 2.4GHz - the
scores/att.v matmuls are inherently ~50% PE-utilized at K=64/M=65);
remaining overheads: ~25us DMA-ring-bound startup, ~20us AG-bound tail,
HAM half-clock stretch around idle windows.

HW gotchas (CoreSim passes all of these; only real HW fails):
  - a single 65-partition DVE copy from PSUM silently corrupts data
  - gpsimd custom-DVE reciprocal misreads PSUM (stage rows in SBUF first)
  - junk "heater" matmuls and fine-grained (16x) AllGathers both regress
"""

import os
import sys
from contextlib import ExitStack

import numpy as np
import ml_dtypes

if "/opt/trn_rl_repo" not in sys.path:
    sys.path.insert(0, "/opt/trn_rl_repo")

import concourse.bass as bass
import concourse.mybir as mybir
import concourse.tile as tile
from concourse import bacc
from concourse import bass_utils

F32 = mybir.dt.float32
BF16 = mybir.dt.bfloat16
P = 128          # SBUF partitions
QT = 512         # q tile (matmul free dim)
KC = 128         # k chunk (psum partition dim)
HS = 64          # head size
KPQ = QT // KC   # k chunks per q tile

N_CORES = 8
PAIRS = [[0, 1], [2, 3], [4, 5], [6, 7]]

B_FULL, T_FULL, C_FULL, H_FULL = 4, 2048, 1024, 16


def build_nc(T=T_FULL, C=C_FULL, HL=H_FULL // 2):
    """Build the SPMD graph for one core (all 8 cores run the same graph).

    Per-core input tensors:
      xT    [C, T] bf16       x[b] transposed
      wqk   [2CL/P, P, C/P, P] bf16  w_attn q|k cols, host-shuffled [f,p,c,m]
      wv_s  [P, C/P, CL] bf16 w_attn v cols, host-shuffled [p,c,m]
      wp_s  [P, C/P, CL] bf16 w_proj cols for this core's output half
      bqk   [2*CL] f32, bv [CL] f32, bp [CL] f32
    Output: out [T, CL] f32.
    """
    CL = HL * HS                 # local width (q, k, v, out-cols each)
    n_cc = C // P                # x feature chunks (8)
    n_f = 2 * CL // P            # q|k f-tiles (4 q then 4 k)
    n_jt = T // QT               # q tiles (4)
    n_kt = T // KC               # k chunks / v t-chunks (16)
    n_pair = HL // 2             # head pairs (4)
    n_half = 2                   # T halves for AG chunking
    TH = T // n_half
    scale = HS ** -0.5

    nc = bacc.Bacc("TRN2", target_bir_lowering=False, debug=False,
                   num_devices=N_CORES)

    xT = nc.dram_tensor("xT", [C, T], BF16, kind="ExternalInput").ap()
    wqk = nc.dram_tensor("wqk", [n_f, P, n_cc, P], BF16,
                         kind="ExternalInput").ap()
    wv_s = nc.dram_tensor("wv_s", [P, n_cc, CL], BF16,
                          kind="ExternalInput").ap()
    wp_s = nc.dram_tensor("wp_s", [P, n_cc, CL], BF16,
                          kind="ExternalInput").ap()
    bqk = nc.dram_tensor("bqk", [P, 2 * CL // P], F32,
                         kind="ExternalInput").ap()
    bv = nc.dram_tensor("bv", [CL], F32, kind="ExternalInput").ap()
    bp = nc.dram_tensor("bp", [CL], F32, kind="ExternalInput").ap()
    out_ext = nc.dram_tensor("out", [T, CL], F32, kind="ExternalOutput").ap()

    with ExitStack() as ctx:
        tc = ctx.enter_context(tile.TileContext(nc))

        persist = ctx.enter_context(tc.tile_pool(name="persist", bufs=1))
        dram = ctx.enter_context(tc.tile_pool(name="dram", bufs=1, space="DRAM"))
        # st 2x2 banks + yp0 + yp1 + qps 2 = 8 banks
        ps = ctx.enter_context(tc.tile_pool(name="ps", bufs=1, space="PSUM"))
        att = ctx.enter_context(tc.tile_pool(name="att", bufs=1))

        # ---- persistent SBUF tiles -----------------------------------
        wqk_sb = [persist.tile([P, n_cc, P], BF16, tag=f"wqk{f}",
                               name=f"wqk{f}") for f in range(n_f)]
        wv_sb = persist.tile([P, n_cc, CL], BF16, tag="wv", name="wv")
        wp_sb = persist.tile([P, n_cc, CL], BF16, tag="wp", name="wp")
        x_sb = [persist.tile([P, T], BF16, tag=f"x{c}", name=f"x{c}")
                for c in range(n_cc)]
        qk_sb = [persist.tile([P, T], BF16, tag=f"qk{f}", name=f"qk{f}")
                 for f in range(n_f)]
        v_sb = [persist.tile([P, HL, HS + 2], BF16, tag=f"v{t}",
                             name=f"v{t}") for t in range(n_kt)]
        oacc = [persist.tile([P, CL], F32, tag=f"oacc{t}", name=f"oacc{t}")
                for t in range(T // P)]
        bqk_sb = persist.tile([P, n_f], F32, tag="bqk", name="bqk_sb")
        bv_bc = persist.tile([P, CL], F32, tag="bv_bc", name="bv_bc")
        bp_bc = persist.tile([P, CL], F32, tag="bp_bc", name="bp_bc")
        ones_f = persist.tile([P, HL, 1], F32, tag="ones_f", name="ones_f")
        # tri[p, g] = 1 where g >= p else 0 (keep-at-or-above-diagonal)
        tri = persist.tile([P, P], BF16, tag="tri", name="tri")

        # ---- input DMAs, in consumption order ------------------------
        # the three DGE rings (sync/gpsimd/scalar) give ~130GB/s aggregate;
        # order transfers so compute can start incrementally: weights for the
        # first atoms, then x by T-quarter so qk/V atoms unlock quarter by
        # quarter
        qs = [nc.sync, nc.gpsimd, nc.scalar]
        # tiny bias rows first (the first qk bias-add gates attention start;
        # bqk is host-shuffled to [p, f] so this is one descriptor per
        # partition, not a 1024-descriptor gather)
        nc.sync.dma_start(bqk_sb[:], bqk)
        bv_row = att.tile([1, CL], F32, tag="brow", bufs=2, name="bv_row")
        nc.sync.dma_start(bv_row[:], bv.rearrange("(o c) -> o c", o=1))
        bp_row = att.tile([1, CL], F32, tag="brow", bufs=2, name="bp_row")
        nc.sync.dma_start(bp_row[:], bp.rearrange("(o c) -> o c", o=1))
        nc.sync.dma_start(wqk_sb[0][:], wqk[0])
        nc.gpsimd.dma_start(wqk_sb[n_jt][:], wqk[n_jt])
        ri = 0
        for q4 in range(4):
            for c in range(n_cc):
                qs[ri % 3].dma_start(
                    x_sb[c][:, q4 * QT:(q4 + 1) * QT],
                    xT[c * P:(c + 1) * P, q4 * QT:(q4 + 1) * QT])
                ri += 1
            if q4 == 0:
                nc.scalar.dma_start(wv_sb[:], wv_s)
        for f in range(n_f):
            if f not in (0, n_jt):
                qs[ri % 3].dma_start(wqk_sb[f][:], wqk[f])
                ri += 1
        nc.scalar.dma_start(wp_sb[:], wp_s)

        nc.gpsimd.partition_broadcast(bv_bc[:], bv_row[:])
        nc.gpsimd.partition_broadcast(bp_bc[:], bp_row[:])
        nc.gpsimd.memset(ones_f[:], 1.0)
        nc.gpsimd.memset(tri[:], 1.0)
        nc.gpsimd.affine_select(
            out=tri[:], in_=tri[:], compare_op=mybir.AluOpType.is_ge,
            fill=0.0, base=0, channel_multiplier=-1, pattern=[[1, P]])

        # ---- AG segments ---------------------------------------------
        # pairs 0-2: two T-half segments. pair 3 (j-order [2,3,1,0]): one
        # T-half for j 2,3 plus two q-tile quarters so the final AG is small
        # and lands right after the shortest j.
        segs = {}   # (pr, j) -> [in_tile, out_tile, col_base, j_set]
        for p in range(n_pair):
            if p < n_pair - 1:
                for h in range(n_half):
                    ti = dram.tile([P, TH], BF16, tag=f"agi{p}_{h}",
                                   name=f"agi{p}_{h}")
                    to = dram.tile([2, P, TH], BF16, tag=f"ago{p}_{h}",
                                   name=f"ago{p}_{h}")
                    for j in (2 * h, 2 * h + 1):
                        segs[(p, j)] = [ti, to, (j % 2) * QT, {2 * h, 2 * h + 1}]
            else:
                ti = dram.tile([P, TH], BF16, tag=f"agi{p}_h1",
                               name=f"agi{p}_h1")
                to = dram.tile([2, P, TH], BF16, tag=f"ago{p}_h1",
                               name=f"ago{p}_h1")
                for j in (2, 3):
                    segs[(p, j)] = [ti, to, (j % 2) * QT, {2, 3}]
                for j in (0, 1):
                    ti = dram.tile([P, QT], BF16, tag=f"agi{p}_q{j}",
                                   name=f"agi{p}_q{j}")
                    to = dram.tile([2, P, QT], BF16, tag=f"ago{p}_q{j}",
                                   name=f"ago{p}_q{j}")
                    segs[(p, j)] = [ti, to, 0, {j}]

        # ---- compute atoms -------------------------------------------
        def v_atom(t):
            """V for t-chunk t: [128 t, CL] + bias, ones col per head."""
            pv = ps.tile([P, CL], F32, tag="qps", bufs=2, name="pv")
            for c in range(n_cc):
                nc.tensor.matmul(
                    pv[:], x_sb[c][:, t * KC:(t + 1) * KC], wv_sb[:, c, :],
                    start=(c == 0), stop=(c == n_cc - 1))
            nc.vector.tensor_copy(v_sb[t][:, :, HS:HS + 1], ones_f[:])
            nc.vector.tensor_add(
                v_sb[t][:, :, 0:HS],
                pv.rearrange("p (h e) -> p h e", e=HS),
                bv_bc.rearrange("p (h e) -> p h e", e=HS))

        def qk_atom(f, t):
            """q/k f-tile x one t-tile of 512: 8 matmuls + bias to SBUF."""
            pq = ps.tile([P, QT], F32, tag="qps", bufs=2, name="pq")
            for c in range(n_cc):
                nc.tensor.matmul(
                    pq[:], wqk_sb[f][:, c, :],
                    x_sb[c][:, t * QT:(t + 1) * QT],
                    start=(c == 0), stop=(c == n_cc - 1))
            nc.vector.tensor_scalar_add(
                qk_sb[f][:, t * QT:(t + 1) * QT], pq[:], bqk_sb[:, f:f + 1])

        proj_pend = []   # prefetched (p, t, [yt0, yt1]) awaiting matmul

        def proj_fetch(p, t):
            j = t // KPQ
            _, to, col_base, _ = segs[(p, j)]
            col = col_base + (t % KPQ) * P
            yts = []
            for gp in range(2):
                yt = att.tile([P, P], BF16, tag="yt", bufs=36, name="yt")
                nc.sync.dma_start(yt[:], to[gp, :, col:col + P])
                yts.append(yt)
            proj_pend.append((p, t, yts))

        def proj_exec(p, t, yts):
            po = ps.tile([P, CL], F32, tag="qps", bufs=2, name="po")
            for gp in range(2):
                nc.tensor.matmul(po[:], yts[gp][:],
                                 wp_sb[:, gp * n_pair + p, :],
                                 start=(gp == 0), stop=(gp == 1))
            if p == 0:
                nc.vector.tensor_add(oacc[t][:], po[:], bp_bc[:])
            else:
                nc.vector.tensor_add(oacc[t][:], oacc[t][:], po[:])
            if p == n_pair - 1:
                nc.sync.dma_start(out_ext[t * P:(t + 1) * P, :], oacc[t][:])

        drain_mode = [False]

        def proj_atom(p, t):
            """Projection tile with 3-deep DMA prefetch; in the drain all
            remaining fetches are issued first so the tail's yt DMAs stream
            while earlier tiles execute."""
            proj_fetch(p, t)
            if len(proj_pend) > 3 and not drain_mode[0]:
                proj_exec(*proj_pend.pop(0))

        def proj_drain():
            while proj_pend:
                proj_exec(*proj_pend.pop(0))

        # ---- filler queue --------------------------------------------
        # (min_pair, min_j, thunk): atom may only be emitted at or after
        # attention position (min_pair, min_j) - proj needs its AG landed,
        # pair-0 q/k and V atoms need their x quarter (position-gated so the
        # attention can start right after x quarter 0 lands).
        filler = []
        filler.append((0, 1, lambda: qk_atom(0, 1)))
        filler.append((0, 1, lambda: qk_atom(n_jt, 1)))
        for t in range(4, 8):
            filler.append((0, 1, (lambda t=t: v_atom(t))))
        for q4 in (2, 3):
            filler.append((0, q4, (lambda t=q4: qk_atom(0, t))))
            filler.append((0, q4, (lambda t=q4: qk_atom(n_jt, t))))
            for t in range(4 * q4, 4 * q4 + 4):
                filler.append((0, q4, (lambda t=t: v_atom(t))))
        for pr in range(1, n_pair):
            for t in range(n_jt):
                filler.append((0, 0, (lambda f=n_jt + pr, t=t: qk_atom(f, t))))
                filler.append((0, 0, (lambda f=pr, t=t: qk_atom(f, t))))
        for p in range(n_pair):
            # positions are (pair, emission-idx). Pairs 0-2: half AGs issue
            # after idx 1 and 3. Pair 3 (j-order [2,3,1,0]): h1 AG at idx 1,
            # quarter AGs after idx 2 and 3 -> those drain in the tail,
            # ready-first (t order 8..15, 4..7, 0..3).
            if p < n_pair - 1:
                ts = list(range(T // P))
            else:
                ts = list(range(8, 16)) + list(range(4, 8)) + list(range(4))
            for t in ts:
                h = t // (TH // P)
                if p < n_pair - 1:
                    mp, mj = (p + 1, 0) if h == 0 else (p + 1, 2)
                else:
                    mp, mj = (p, 2) if h == 1 else (p, n_jt)
                filler.append((mp, mj, (lambda p=p, t=t: proj_atom(p, t))))

        def pop_filler(pr, j):
            for idx, (mp, mj, thunk) in enumerate(filler):
                if (mp, mj) <= (pr, j):
                    filler.pop(idx)
                    thunk()
                    return True
            return False

        # ---- attention -----------------------------------------------
        def attention_pair(pr, js):
            """Both heads of pair pr; scores^T [k, q] stripes, 2-deep
            pipelined st -> exp -> av so PE never waits on ACT."""
            kT = qk_sb[n_pair + pr]
            qTt = qk_sb[pr]
            done_j = set()
            ag_done = set()
            for idx, j in enumerate(js):
                yps = {rr: ps.tile([P, QT], F32, tag=f"yp{rr}", bufs=1,
                                   name=f"yp{rr}") for rr in range(2)}
                imax = KPQ * j + KPQ
                # fillers here keep the PE busy while the previous j's
                # normalize chain releases the yp PSUM banks
                pop_filler(pr, idx)
                pop_filler(pr, idx)
                pend = []   # pipelined (i, off, pt) awaiting av

                def av(iv, offv, ptv):
                    for rr in range(2):
                        nc.tensor.matmul(
                            yps[rr][0:HS + 1, offv:QT],
                            v_sb[iv][:, 2 * pr + rr, 0:HS + 1],
                            ptv[:, rr, offv:QT],
                            start=(iv == 0), stop=(iv == imax - 1))

                for i in range(imax):
                    diag = (i // KPQ == j)
                    # causally trim diagonal chunks to q >= i*KC
                    off = KC * (i % KPQ) if diag else 0
                    st = ps.tile([P, 2, QT], F32, tag="st", bufs=2,
                                 name="st")
                    for rr in range(2):
                        ro = HS * rr
                        nc.tensor.matmul(
                            st[:, rr, off:QT],
                            kT[ro:ro + HS, i * KC:(i + 1) * KC],
                            qTt[ro:ro + HS, j * QT + off:(j + 1) * QT],
                            start=True, stop=True)
                    pt = att.tile([P, 2, QT], BF16, tag="pt", bufs=6,
                                  name="pt")
                    nc.scalar.activation(
                        pt[:, :, off:QT], st[:, :, off:QT],
                        mybir.ActivationFunctionType.Exp, scale=scale)
                    if diag:
                        for rr in range(2):
                            # zero above the diagonal in the leading
                            # 128x128 triangle, in place
                            nc.vector.tensor_mul(
                                pt[:, rr, off:off + KC],
                                pt[:, rr, off:off + KC], tri[:])
                    pend.append((i, off, pt))
                    if len(pend) > 3:
                        av(*pend.pop(0))
                    if i % 2 == 1:
                        pop_filler(pr, idx)
                while pend:
                    av(*pend.pop(0))
                for rr in range(2):
                    ro = HS * rr
                    # custom-DVE recip misreads PSUM AND misreads inputs at
                    # a nonzero base partition: stage the sum row into its
                    # own base-0 SBUF tile first. Keep this chain at exactly
                    # these 4 ops - freeing yps earlier via a full-bank copy
                    # (5-op chain) collapses the clock on HW (2x measured).
                    row = att.tile([1, QT], F32, tag="row", bufs=3,
                                   name="row")
                    nc.vector.tensor_copy(row[:], yps[rr][HS:HS + 1, :])
                    rec = att.tile([1, QT], F32, tag="rec", bufs=3,
                                   name="rec")
                    nc.vector.reciprocal_approx_fast(rec[:], row[:])
                    rb = att.tile([HS, QT], F32, tag="rb", bufs=3, name="rb")
                    nc.gpsimd.partition_broadcast(rb[:], rec[:])
                    yn = att.tile([HS, QT], BF16, tag="yn", bufs=4,
                                  name="yn")
                    nc.vector.tensor_mul(yn[:], yps[rr][0:HS, :], rb[:])
                    ti, _, col_base, _ = segs[(pr, j)]
                    nc.sync.dma_start(
                        ti[ro:ro + HS, col_base:col_base + QT], yn[:])
                done_j.add(j)
                ti, to, _, j_set = segs[(pr, j)]
                if j_set <= done_j and id(ti) not in ag_done:
                    ag_done.add(id(ti))
                    nc.gpsimd.collective_compute(
                        "AllGather", mybir.AluOpType.bypass,
                        replica_groups=PAIRS,
                        ins=[ti.opt()], outs=[to.opt()])

        # ---- schedule ------------------------------------------------
        # minimal upfront: just what pair-0 j=0 needs (x quarter 0 derived);
        # everything else flows in through the filler queue
        qk_atom(0, 0)
        qk_atom(n_jt, 0)
        for t in range(4):
            v_atom(t)

        for pr in range(n_pair):
            js = [2, 3, 1, 0] if pr == n_pair - 1 else list(range(n_jt))
            attention_pair(pr, js)
        # drain remaining fillers (last AG half's projection tiles):
        # issue every remaining yt fetch before executing any of them
        drain_mode[0] = True
        while pop_filler(n_pair - 1, n_jt):
            pass
        proj_drain()

    nc.compile()
    return nc


def shard_inputs(x, w_attn, b_attn, w_proj, b_proj):
    """Slice/transpose/shuffle full inputs into 8 per-core input maps."""
    Bq, T, C = x.shape
    CL = C // 2
    n_cc = C // P
    n_f = 2 * CL // P
    bf = ml_dtypes.bfloat16
    in_maps = []
    for i in range(N_CORES):
        b, g = i // 2, i % 2
        sl = slice(CL * g, CL * (g + 1))
        wq = w_attn[:, sl]
        wk = w_attn[:, C + CL * g:C + CL * (g + 1)]
        wvv = w_attn[:, 2 * C + CL * g:2 * C + CL * (g + 1)]
        wqk = np.concatenate([wq, wk], axis=1)          # [C, 2CL]
        # [C, 2CL] -> [f, p, c, m]: row r = c*128+p, col = f*128+m
        wqk_s = np.ascontiguousarray(
            wqk.reshape(n_cc, P, n_f, P).transpose(2, 1, 0, 3)).astype(bf)
        wv_shuf = np.ascontiguousarray(
            wvv.reshape(n_cc, P, CL).transpose(1, 0, 2)).astype(bf)
        wp_shuf = np.ascontiguousarray(
            w_proj[:, sl].reshape(n_cc, P, CL).transpose(1, 0, 2)).astype(bf)
        in_maps.append({
            "xT": np.ascontiguousarray(x[b].T).astype(bf),
            "wqk": wqk_s,
            "wv_s": wv_shuf,
            "wp_s": wp_shuf,
            "bqk": np.ascontiguousarray(
                np.concatenate([b_attn[sl],
                                b_attn[C + CL * g:C + CL * (g + 1)]])
                .reshape(n_f, P).T),
            "bv": np.ascontiguousarray(b_attn[2 * C + CL * g:2 * C + CL * (g + 1)]),
            "bp": np.ascontiguousarray(b_proj[sl]),
        })
    return in_maps


def gather_outputs(results, B, T, C):
    CL = C // 2
    out = np.empty((B, T, C), dtype=np.float32)
    for i in range(N_CORES):
        b, g = i // 2, i % 2
        out[b, :, CL * g:CL * (g + 1)] = results[i]["out"]
    return out


_NC_CACHE = {}


def get_nc(T, C):
    key = (T, C)
    if key not in _NC_CACHE:
        _NC_CACHE[key] = build_nc(T=T, C=C, HL=C // HS // 2)
    return _NC_CACHE[key]


def kernel(x, w_attn, b_attn, w_proj, b_proj):
    x = np.asarray(x, dtype=np.float32)
    w_attn = np.asarray(w_attn, dtype=np.float32)
    b_attn = np.asarray(b_attn, dtype=np.float32)
    w_proj = np.asarray(w_proj, dtype=np.float32)
    b_proj = np.asarray(b_proj, dtype=np.float32)

    Bq, T, C = x.shape
    nc = get_nc(T, C)

    in_maps = shard_inputs(x, w_attn, b_attn, w_proj, b_proj)
    trace = os.environ.get("KERNEL_TRACE", "0") == "1"
    res = bass_utils.run_bass_kernel_spmd(
        nc, in_maps, core_ids=list(range(N_CORES)), trace=trace)
    if trace and res.exec_time_ns is not None:
        print(f"HW exec time: {res.exec_time_ns} ns", flush=True)
        kernel.last_exec_time_ns = res.exec_time_ns
        kernel.last_results = res
    return gather_outputs(res.results, Bq, T, C)
